# revision 11
# baseline (speedup 1.0000x reference)
import math
from contextlib import ExitStack

import numpy as np

import concourse.bass as bass
from concourse import mybir
from concourse.bass_utils import run_bass_kernel_spmd

K = 5
PAD = K // 2
TAU = 0.1
ALPHA = 2.0
BETA = 1.0
MAXH = math.log(K * K - 1)

_CACHED_NC = None


def _build_bass():
    """Raw-Bass SPMD program (one core): s = Ws@fs3 [64,4096] and
    tt = Wt@ft [64,1024] via PE matmuls; explicit per-chunk semaphores."""
    nc = bass.Bass(target_bir_lowering=False)
    f32 = mybir.dt.float32
    fs3 = nc.dram_tensor("fs3", [256, 4096], f32, kind="ExternalInput")
    ft = nc.dram_tensor("ft", [512, 1024], f32, kind="ExternalInput")
    wpk = nc.dram_tensor("Wpack", [128, 6, 64], f32, kind="ExternalInput")
    s_out = nc.dram_tensor("s_out", [64, 4096], f32, kind="ExternalOutput")
    tt_out = nc.dram_tensor("tt_out", [64, 1024], f32, kind="ExternalOutput")

    fs3_t = fs3[:, :].rearrange("(a p) n -> p a n", p=128)  # [128, 2, 4096]
    ft_t = ft[:, :].rearrange("(a p) n -> p a n", p=128)    # [128, 4, 1024]

    with ExitStack() as ctx:
        w = ctx.enter_context(nc.sbuf_tensor("w", [128, 6, 64], f32))
        xs = [ctx.enter_context(nc.sbuf_tensor(f"xs{i}", [128, 2, 512], f32))
              for i in range(8)]
        xt = [ctx.enter_context(nc.sbuf_tensor(f"xt{i}", [128, 4, 512], f32))
              for i in range(2)]
        o = [ctx.enter_context(nc.sbuf_tensor(f"o{i}", [64, 512], f32))
             for i in range(10)]
        ps = [ctx.enter_context(nc.psum_tensor(f"ps{i}", [64, 512], f32))
              for i in range(8)]
        wsem = ctx.enter_context(nc.semaphore("wsem"))
        din = [ctx.enter_context(nc.semaphore(f"din{i}")) for i in range(10)]
        pe_sem = ctx.enter_context(nc.semaphore("pe_sem"))
        dve_sem = ctx.enter_context(nc.semaphore("dve_sem"))
        dout = ctx.enter_context(nc.semaphore("dout"))
        block = ctx.enter_context(nc.Block())

        @block.gpsimd
        def _(gpsimd):
            gpsimd.dma_start(w[:, :, :], wpk[:, :, :]).then_inc(wsem, 16)
            for i in range(8):
                gpsimd.dma_start(
                    xs[i][:, :, :], fs3_t[:, :, i * 512:(i + 1) * 512]
                ).then_inc(din[i], 16)
            for j in range(2):
                gpsimd.dma_start(
                    xt[j][:, :, :], ft_t[:, :, j * 512:(j + 1) * 512]
                ).then_inc(din[8 + j], 16)
            for i in range(8):
                gpsimd.wait_ge(dve_sem, i + 1)
                gpsimd.dma_start(
                    s_out[:, i * 512:(i + 1) * 512], o[i][:, :]
                ).then_inc(dout, 16)
            for j in range(2):
                gpsimd.wait_ge(dve_sem, 9 + j)
                gpsimd.dma_start(
                    tt_out[:, j * 512:(j + 1) * 512], o[8 + j][:, :]
                ).then_inc(dout, 16)

        @block.tensor
        def _(tensor):
            tensor.wait_ge(wsem, 16)
            for i in range(8):
                tensor.wait_ge(din[i], 16)
                nc.tensor.matmul(ps[i % 8][:, :], w[:, 0, :], xs[i][:, 0, :],
                                 start=True, stop=False)
                nc.tensor.matmul(ps[i % 8][:, :], w[:, 1, :], xs[i][:, 1, :],
                                 start=False, stop=True).then_inc(pe_sem, 1)
            for j in range(2):
                tensor.wait_ge(din[8 + j], 16)
                tensor.wait_ge(dve_sem, j + 1)  # bank reuse WAR
                for kt in range(4):
                    mm = nc.tensor.matmul(ps[j][:, :], w[:, 2 + kt, :],
                                          xt[j][:, kt, :],
                                          start=(kt == 0), stop=(kt == 3))
                mm.then_inc(pe_sem, 1)

        @block.vector
        def _(vector):
            for i in range(10):
                vector.wait_ge(pe_sem, i + 1)
                nc.vector.tensor_copy(o[i][:, :], ps[i % 8][:, :]).then_inc(
                    dve_sem, 1)
    return nc


def _upsample2x(x):
    """Bilinear x2 upsample, half-pixel centers (jax.image.resize 'bilinear'),
    along the last two axes. x: [..., H, W] -> [..., 2H, 2W]."""
    def up1(a):  # along last axis
        n = a.shape[-1]
        left = np.concatenate([a[..., :1], a[..., :-1]], axis=-1)
        right = np.concatenate([a[..., 1:], a[..., -1:]], axis=-1)
        out = np.empty(a.shape[:-1] + (2 * n,), dtype=a.dtype)
        out[..., 0::2] = 0.25 * left + 0.75 * a
        out[..., 1::2] = 0.75 * a + 0.25 * right
        return out
    x = up1(x)
    x = np.swapaxes(up1(np.swapaxes(x, -1, -2)), -1, -2)
    return x


def _logp(f):
    """f: [B, d, H, W] -> [B, H*W, 24] log_softmax of local affinities."""
    B, d, H, W = f.shape
    nrm = np.sqrt(np.sum(f * f, axis=1, keepdims=True))
    fn = f / np.maximum(nrm, 1e-12)
    fp = np.pad(fn, ((0, 0), (0, 0), (PAD, PAD), (PAD, PAD)), mode="reflect")
    center = K * K // 2
    affs = []
    for i in range(K):
        for j in range(K):
            if i * K + j == center:
                continue
            affs.append(np.einsum("bdhw,bdhw->bhw", fn, fp[:, :, i:i + H, j:j + W]))
    aff = np.stack(affs, axis=-1).reshape(B, H * W, K * K - 1)
    x = aff / TAU
    m = np.max(x, axis=-1, keepdims=True)
    e = np.exp(x - m)
    return x - m - np.log(np.sum(e, axis=-1, keepdims=True))


def _sobel_mag(x):
    """x: [B, C, H, W] -> [B, H*W]; 3x3 sobel on channel-mean, zero 'SAME' pad,
    per-image min-max normalized."""
    xm = np.mean(x, axis=1)  # [B, H, W]
    p = np.pad(xm, ((0, 0), (1, 1), (1, 1)))
    kx = np.array([[-1., 0., 1.], [-2., 0., 2.], [-1., 0., 1.]], dtype=x.dtype)
    ky = np.array([[-1., -2., -1.], [0., 0., 0.], [1., 2., 1.]], dtype=x.dtype)
    B, H, W = xm.shape
    gx = np.zeros_like(xm)
    gy = np.zeros_like(xm)
    for i in range(3):
        for j in range(3):
            sl = p[:, i:i + H, j:j + W]
            if kx[i, j] != 0:
                gx += kx[i, j] * sl
            if ky[i, j] != 0:
                gy += ky[i, j] * sl
    m = np.sqrt(gx * gx + gy * gy)
    mn = m.min(axis=(1, 2), keepdims=True)
    mx = m.max(axis=(1, 2), keepdims=True)
    m = (m - mn) / (mx - mn + 1e-6)
    return m.reshape(B, -1)


def kernel(fs3, ft, Ws, Wt):
    global _CACHED_NC
    fs3 = np.asarray(fs3, np.float32)
    ft = np.asarray(ft, np.float32)
    Ws = np.asarray(Ws, np.float32)
    Wt = np.asarray(Wt, np.float32)
    B, _, H, W = fs3.shape  # 8, 256, 64, 64

    if _CACHED_NC is None:
        _CACHED_NC = _build_bass()
    nc = _CACHED_NC

    WsT = Ws.T.reshape(2, 128, 64)
    WtT = Wt.T.reshape(4, 128, 64)
    Wpack = np.ascontiguousarray(
        np.concatenate([WsT, WtT], axis=0).transpose(1, 0, 2))  # [128, 6, 64]
    in_maps = []
    for b in range(B):
        in_maps.append({
            "fs3": np.ascontiguousarray(fs3[b].reshape(256, H * W)),
            "ft": np.ascontiguousarray(ft[b].reshape(512, 32 * 32)),
            "Wpack": Wpack,
        })
    res = run_bass_kernel_spmd(nc, in_maps, core_ids=list(range(8)))
    s = np.stack([res.results[b]["s_out"] for b in range(B)]).reshape(B, 64, H, W)
    tt = np.stack([res.results[b]["tt_out"] for b in range(B)]).reshape(B, 64, 32, 32)
    # bilinear resize commutes with the 1x1 conv: t = resize(Wt @ ft)
    t = _upsample2x(tt)

    t_logp = _logp(t)
    s_logp = _logp(s)
    t_p = np.exp(t_logp)
    ent = -np.sum(t_p * t_logp, axis=-1)
    w_conf = np.clip(1.0 - ent / (MAXH + 1e-12), 0.0, 1.0)
    w_edge = _sobel_mag(t)
    w = w_conf ** ALPHA * (1.0 + BETA * w_edge)
    m = np.pad(np.ones((B, H - 2 * PAD, W - 2 * PAD), dtype=fs3.dtype),
               ((0, 0), (PAD, PAD), (PAD, PAD)))
    w = w * m.reshape(B, -1)
    kl = np.sum(t_p * (t_logp - s_logp), axis=-1)
    out = np.sum(w * kl) / (np.sum(w) + 1e-6)
    return np.float32(out)
